# revision 9
# baseline (speedup 1.0000x reference)
"""Bahdanau additive attention on 8 Trainium2 NeuronCores.

Data-parallel over batch: core c handles batches [4c, 4c+4).
Per batch b:
  ep[k,t]   = sum_h Ua[k,h] * enc[b,t,h]        (fp32r PE matmuls, k on PSUM partitions)
  z[k,t]    = tanh(ep[k,t] + hp[b,k])           (ScalarE, hp as per-partition bias)
  e[t]      = sum_k va[k] * z[k,t]              (M=1 fp32r PE matmuls)
  attn      = softmax(e + (mask-1)*1e30)        (DVE/ScalarE on [1,T])
  ctx[h]    = sum_t attn[t] * enc[b,t,h]        (M=1 fp32r PE matmuls, enc natural layout)
hp[b,k] = sum_h Wa[k,h] * h_t[b,h] is computed on device from host-transposed Wa.
Host prep: transposes (Ua.T, Wa.T, h_t.T, enc.transpose) to give every DMA a
contiguous partition-major layout.
"""

import numpy as np

import concourse.bass as bass
import concourse.tile as tile
from concourse import bacc, mybir

dt = mybir.dt
AF = mybir.ActivationFunctionType

B, T, H = 32, 1024, 1024
NCORES = 8
BL = B // NCORES          # batches per core
P = 128                   # partitions
NT = 512                  # matmul free-dim chunk (one PSUM bank of fp32)
KT = H // P               # k-tiles (output rows of ep)
HT = H // P               # h-tiles (contraction)
TT = T // P               # t-tiles (partition tiles of natural enc)
TC = T // NT              # t chunks per batch

_CACHE = {}


def _build_nc():
    nc = bacc.Bacc("TRN2", target_bir_lowering=False, debug=False)

    encT_d = nc.dram_tensor("encT", [BL, H, T], dt.float32r, kind="ExternalInput").ap()
    encn_d = nc.dram_tensor("encn", [BL, T, H], dt.float32r, kind="ExternalInput").ap()
    uaT_d = nc.dram_tensor("uaT", [H, H], dt.float32r, kind="ExternalInput").ap()
    waT_d = nc.dram_tensor("waT", [H, H], dt.float32r, kind="ExternalInput").ap()
    htT_d = nc.dram_tensor("htT", [H, BL], dt.float32r, kind="ExternalInput").ap()
    va_d = nc.dram_tensor("va", [H], dt.float32r, kind="ExternalInput").ap()
    mask_d = nc.dram_tensor("mask", [BL, T], dt.uint8, kind="ExternalInput").ap()

    ctx_d = nc.dram_tensor("ctx", [BL, H], dt.float32, kind="ExternalOutput").ap()
    attn_d = nc.dram_tensor("attn", [BL, T], dt.float32, kind="ExternalOutput").ap()

    with tile.TileContext(nc) as tc:
        from contextlib import ExitStack

        with ExitStack() as st:
            wpool = st.enter_context(tc.tile_pool(name="weights", bufs=1))
            etpool = st.enter_context(tc.tile_pool(name="encT", bufs=12))
            natpool = st.enter_context(tc.tile_pool(name="nat", bufs=12))
            thpool = st.enter_context(tc.tile_pool(name="tanh", bufs=4))
            smpool = st.enter_context(tc.tile_pool(name="small", bufs=1))
            pmain = st.enter_context(tc.tile_pool(name="pmain", bufs=3, space="PSUM"))
            pe_ps = st.enter_context(tc.tile_pool(name="pe", bufs=2, space="PSUM"))
            pctx = st.enter_context(tc.tile_pool(name="pctx", bufs=1, space="PSUM"))
            php = st.enter_context(tc.tile_pool(name="php", bufs=1, space="PSUM"))

            # ---- persistent weights / vectors ----
            waT_sb = []
            for ht in range(HT):
                w = wpool.tile([P, H], dt.float32r, tag=f"waT{ht}")
                nc.sync.dma_start(w[:], waT_d[ht * P:(ht + 1) * P, :])
                waT_sb.append(w)
            uaT_sb = []
            for ht in range(HT):
                w = wpool.tile([P, H], dt.float32r, tag=f"uaT{ht}")
                nc.sync.dma_start(w[:], uaT_d[ht * P:(ht + 1) * P, :])
                uaT_sb.append(w)
            htT_sb = wpool.tile([P, HT, BL], dt.float32r, tag="htT")
            nc.sync.dma_start(htT_sb[:], htT_d.rearrange("(ht p) b -> p ht b", p=P))
            va_sb = wpool.tile([P, KT], dt.float32r, tag="va")
            nc.sync.dma_start(va_sb[:], va_d.rearrange("(kt p) -> p kt", p=P))
            ones_sb = wpool.tile([1, 1], dt.float32, tag="ones")
            nc.vector.memset(ones_sb[:], 1.0)
            negbig = wpool.tile([1, 1], dt.float32, tag="negbig")
            nc.vector.memset(negbig[:], -1e30)
            hp_sb = wpool.tile([P, KT, BL], dt.float32, tag="hp")

            def emit_hp(kt):
                ps = php.tile([P, BL], dt.float32, tag="hp")
                for ht in range(HT):
                    nc.tensor.matmul(
                        ps[:], waT_sb[ht][:, kt * P:(kt + 1) * P], htT_sb[:, ht, :],
                        start=(ht == 0), stop=(ht == HT - 1))
                nc.vector.tensor_copy(hp_sb[:, kt, :], ps[:])

            def make_tail(bi, e_sb, mask_f, nat_t):
                def emit_tail():
                    # mask: e += (m - 1) * 1e30  (m in {0,1}; no-op where m == 1)
                    me = smpool.tile([1, T], dt.float32, tag="me")
                    nc.scalar.activation(me[:], mask_f[:], AF.Identity,
                                         bias=negbig[:], scale=1e30)
                    nc.vector.tensor_add(e_sb[:], e_sb[:], me[:])
                    # softmax over free dim
                    nm = smpool.tile([1, 1], dt.float32, tag="nm")
                    nc.vector.tensor_reduce(nm[:], e_sb[:], axis=mybir.AxisListType.X,
                                            op=mybir.AluOpType.max, negate=True)
                    ex = smpool.tile([1, T], dt.float32, tag="ex")
                    ssum = smpool.tile([1, 1], dt.float32, tag="ssum")
                    nc.scalar.activation(ex[:], e_sb[:], AF.Exp, bias=nm[:],
                                         accum_out=ssum[:])
                    rinv = smpool.tile([1, 1], dt.float32, tag="rinv")
                    nc.vector.reciprocal(rinv[:], ssum[:])
                    attn_sb = smpool.tile([1, T], dt.float32, tag="attn")
                    nc.vector.tensor_scalar_mul(attn_sb[:], ex[:], rinv[:])
                    nc.sync.dma_start(attn_d[bi:bi + 1, :], attn_sb[:])
                    # transpose attn into partitions: [1,T] -> [P, TT]
                    atp = pctx.tile([P, TT], dt.float32, tag="attnT")
                    for tt in range(TT):
                        nc.tensor.matmul(
                            atp[:, tt:tt + 1], attn_sb[:, tt * P:(tt + 1) * P],
                            ones_sb[:], start=True, stop=True)
                    attnT = smpool.tile([P, TT], dt.float32r, tag="attnTsb")
                    nc.vector.tensor_copy(attnT[:], atp[:])
                    # context: ctx[h] = sum_t attn[t] enc[t, h]
                    ctx_sb = smpool.tile([1, H], dt.float32, tag="ctx")
                    for kc in range(H // NT):
                        cp = pctx.tile([1, NT], dt.float32, tag="ctx")
                        for tt in range(TT):
                            nc.tensor.matmul(
                                cp[:], attnT[:, tt:tt + 1],
                                nat_t[tt][:, kc * NT:(kc + 1) * NT],
                                start=(tt == 0), stop=(tt == TT - 1))
                        nc.vector.tensor_copy(ctx_sb[:, kc * NT:(kc + 1) * NT], cp[:])
                    nc.sync.dma_start(ctx_d[bi:bi + 1, :], ctx_sb[:])
                return emit_tail

            pending_tail = None
            for bi in range(BL):
                encT_t = []
                for ht in range(HT):
                    t_ = etpool.tile([P, T], dt.float32r, tag="encT")
                    nc.sync.dma_start(t_[:], encT_d[bi, ht * P:(ht + 1) * P, :])
                    encT_t.append(t_)
                nat_t = []
                for tt in range(TT):
                    t_ = natpool.tile([P, H], dt.float32r, tag="nat")
                    nc.sync.dma_start(t_[:], encn_d[bi, tt * P:(tt + 1) * P, :])
                    nat_t.append(t_)
                mask_f = smpool.tile([1, T], dt.float32, tag="mask", bufs=2)
                nc.gpsimd.dma_start(mask_f[:], mask_d[bi:bi + 1, :])

                e_ps = [pe_ps.tile([1, NT], dt.float32, tag="e", name=f"e_ps{_}")
                         for _ in range(TC)]
                pending_emm = None
                gidx = 0
                for kt in range(KT):
                    if bi == 0:
                        emit_hp(kt)
                    for tcc in range(TC):
                        if gidx == 3 and pending_tail is not None:
                            pending_tail()
                            pending_tail = None
                        ps = pmain.tile([P, NT], dt.float32, tag="big")
                        for ht in range(HT):
                            nc.tensor.matmul(
                                ps[:], uaT_sb[ht][:, kt * P:(kt + 1) * P],
                                encT_t[ht][:, tcc * NT:(tcc + 1) * NT],
                                start=(ht == 0), stop=(ht == HT - 1))
                        th = thpool.tile([P, NT], dt.float32r, tag="th")
                        nc.scalar.activation(th[:], ps[:], AF.Tanh,
                                             bias=hp_sb[:, kt, bi:bi + 1])
                        if pending_emm is not None:
                            pending_emm()
                        def make_emm(kt=kt, tcc=tcc, th=th):
                            def emm():
                                nc.tensor.matmul(
                                    e_ps[tcc][:], va_sb[:, kt:kt + 1], th[:],
                                    start=(kt == 0), stop=(kt == KT - 1))
                            return emm
                        pending_emm = make_emm()
                        gidx += 1
                pending_emm()
                # e chunks -> SBUF (frees e psum slots early)
                e_sb = smpool.tile([1, T], dt.float32, tag="e_sb", bufs=2)
                for tcc in range(TC):
                    nc.vector.tensor_copy(e_sb[:, tcc * NT:(tcc + 1) * NT],
                                          e_ps[tcc][:])
                pending_tail = make_tail(bi, e_sb, mask_f, nat_t)
            pending_tail()

    nc.compile()
    return nc


def _get_runner():
    if "runner" in _CACHE:
        return _CACHE["runner"]

    import jax
    import jax.numpy as jnp  # noqa: F401
    from jax.sharding import Mesh, PartitionSpec
    from jax.experimental.shard_map import shard_map
    from concourse import bass2jax
    from concourse import mybir as _mb

    nc = _build_nc()
    bass2jax.install_neuronx_cc_hook()

    partition_name = (nc.partition_id_tensor.name
                      if nc.partition_id_tensor else None)
    in_names, out_names, out_avals, zero_outs = [], [], [], []
    for alloc in nc.m.functions[0].allocations:
        if not isinstance(alloc, _mb.MemoryLocationSet):
            continue
        name = alloc.memorylocations[0].name
        if alloc.kind == "ExternalInput":
            if name != partition_name:
                in_names.append(name)
        elif alloc.kind == "ExternalOutput":
            out_names.append(name)
            shape = tuple(alloc.tensor_shape)
            npdt = _mb.dt.np(alloc.dtype)
            out_avals.append(jax.core.ShapedArray(shape, npdt))
            zero_outs.append(np.zeros(shape, npdt))
    n_params = len(in_names)
    n_outs = len(out_names)
    all_in_names = in_names + out_names
    if partition_name is not None:
        all_in_names = all_in_names + [partition_name]
    donate = tuple(range(n_params, n_params + n_outs))

    def _body(*args):
        operands = list(args)
        if partition_name is not None:
            operands.append(bass2jax.partition_id_tensor())
        outs = bass2jax._bass_exec_p.bind(
            *operands,
            out_avals=tuple(out_avals),
            in_names=tuple(all_in_names),
            out_names=tuple(out_names),
            lowering_input_output_aliases=(),
            sim_require_finite=True,
            sim_require_nnan=True,
            nc=nc,
        )
        return tuple(outs)

    devices = jax.devices()[:NCORES]
    mesh = Mesh(np.asarray(devices), ("core",))
    in_specs = (PartitionSpec("core"),) * (n_params + n_outs)
    out_specs = (PartitionSpec("core"),) * n_outs
    sharded = jax.jit(
        shard_map(_body, mesh=mesh, in_specs=in_specs, out_specs=out_specs,
                  check_rep=False),
        donate_argnums=donate, keep_unused=True)

    def run(in_maps):
        concat_in = [
            np.concatenate([np.asarray(m[name]) for m in in_maps], axis=0)
            for name in in_names
        ]
        concat_zeros = [
            np.zeros((NCORES * z.shape[0], *z.shape[1:]), z.dtype)
            for z in zero_outs
        ]
        out_arrs = sharded(*concat_in, *concat_zeros)
        return [
            {name: np.asarray(out_arrs[i]).reshape(NCORES, *out_avals[i].shape)[c]
             for i, name in enumerate(out_names)}
            for c in range(NCORES)
        ]

    _CACHE["runner"] = run
    return run


def _make_in_maps(inputs):
    h_t = np.asarray(inputs["h_t"], dtype=np.float32)
    enc_out = np.asarray(inputs["enc_out"], dtype=np.float32)
    src_mask = np.asarray(inputs["src_mask"])
    Wa = np.asarray(inputs["Wa"], dtype=np.float32)
    Ua = np.asarray(inputs["Ua"], dtype=np.float32)
    va = np.asarray(inputs["va"], dtype=np.float32)

    uaT = np.ascontiguousarray(Ua.T)
    waT = np.ascontiguousarray(Wa.T)
    htT = np.ascontiguousarray(h_t.T)                        # [H, B]
    encT = np.ascontiguousarray(enc_out.transpose(0, 2, 1))  # [B, H, T]
    mask_u8 = np.ascontiguousarray(src_mask.astype(np.uint8))

    in_maps = []
    for c in range(NCORES):
        sl = slice(c * BL, (c + 1) * BL)
        in_maps.append({
            "encT": encT[sl],
            "encn": np.ascontiguousarray(enc_out[sl]),
            "uaT": uaT,
            "waT": waT,
            "htT": np.ascontiguousarray(htT[:, sl]),
            "va": va,
            "mask": mask_u8[sl],
        })
    return in_maps


def kernel(h_t, enc_out, src_mask, Wa, Ua, va):
    in_maps = _make_in_maps({
        "h_t": h_t, "enc_out": enc_out, "src_mask": src_mask,
        "Wa": Wa, "Ua": Ua, "va": va,
    })
    run = _get_runner()
    results = run(in_maps)
    context = np.concatenate([r["ctx"] for r in results], axis=0)
    attn = np.concatenate([r["attn"] for r in results], axis=0)
    return context, attn
